# revision 9
# baseline (speedup 1.0000x reference)
"""Chamfer distance loss on 8 Trainium2 cores — fused single-pass reduction
via a custom DVE op.

Strategy (hardcoded for B=16, N=M=4096, D=3 fp32 inputs):
  - Data-parallel over batch: core c handles batches {2c, 2c+1}; each core
    returns a partial scalar sum; host adds the 8 partials and divides by B.
  - Per batch, the (4096 x 4096) negated squared-distance matrix is produced
    once (single orientation) on the tensor engine as an augmented matmul
    with K=24 (3-level bf16 split per fp32 factor, ~1e-7 distance error,
    full bf16 PE rate), into [128, 16, 128] fp32 PSUM tiles (4 banks,
    double-buffered = all 8 banks).
  - Both reductions come from ONE pass per matrix element via a custom DVE
    op MAXBOTH_ANT: out = max(in0, in1), accum_out = max(s0, rowmax(in0)).
    The accumulator folds Src0 ONLY — the lowered uop program's accumulator
    stage has its body input rewired to the Src0 delay lane (probe-validated
    on HW).  in0 streams the fresh distance tile, in1/out stream a running
    column-max (bf16), so the row side rides the accumulator and the col
    side the elementwise body.
  - Two block paths balance DVE against ScalarE:
      * fp32-direct: DVE reads the PSUM tile at 1 position/cycle.
      * bf16-staged: ScalarE converts the PSUM tile to bf16 in SBUF; the
        DVE consumes the whole staged row in one 1x instruction.
    NOTE: a 2X_1PORT program for MAXBOTH exists (uops_2x) and its BODY is
    HW-correct at 2 positions/cycle, but the perf-mode accumulator readout
    is firmware-incomplete (custom-dve design doc T1): accum_out returns
    garbage whenever byte-36 perf_max != 0, so all instructions stay 1x.
    Single-src scan-style workarounds (running-max emitted via the write
    path) hang the engine at perf modes — same root cause.
  - Col-side epilogue per batch: PE transposes the final bf16 colmax
    directly (bf16 identity, bf16 PSUM tiles), tensor_reduce(max, axis=X)
    extracts per-column minima; a K=128 matmul against ones folds
    partitions into the [1,1] output.
  - PE row groups alternate between partition offsets 0 and 32 (operands
    replicated host-side) so LDWEIGHTS overlaps in-flight matmuls.
"""

import sys

if "/opt/trn_rl_repo" not in sys.path:
    sys.path.insert(0, "/opt/trn_rl_repo")

import copy
import time

import numpy as np
import ml_dtypes

BF16 = ml_dtypes.bfloat16

B, N, D = 16, 4096, 3
NCORES = 8
BPC = B // NCORES          # batches per core
KAUG = 24                  # augmented contraction dim (hi/mid/lo split)
PT = 128                   # stationary points per matmul (psum partitions)
FT = 512                   # moving points per matmul
CH = 2048                  # columns per psum tile (4 banks)
NI = N // PT               # 32 stationary row-blocks per batch
GR = BPC * 2               # 4 groups of [KAUG, N]: (batch, side)

# row-block indices on the fp32-direct path (DVE streams PSUM at 1x);
# the rest take the bf16-2x path (ScalarE stages, DVE consumes at 2x).
# 7 evenly spread direct blocks balance DVE against ScalarE.
NDIR = 7
DIRECT_BLOCKS = frozenset(round(k * NI / NDIR) for k in range(NDIR))
BF16_BLOCKS = frozenset(range(NI)) - DIRECT_BLOCKS

IDENT = np.eye(PT, dtype=np.float32)

_MAXBOTH_NAME = "MAXBOTH_ANT"


def _make_2x(uops):
    """Hand-authored 2X_1PORT program for MAXBOTH (see module docstring)."""
    from concourse.dve_uop import (
        UopDpConfig, AluInp, DelayInp, InpSel, OutSel, OutPath, AluOp,
    )

    A, PD, PAO = AluInp, DelayInp.PREV_DELAY, DelayInp.PREV_ALU_OUT
    MAX, BYP = AluOp.MAX, AluOp.BYPASS

    def dp(op, src0, src1, delay_map, out_a=0):
        delay = [DelayInp.PREV_ALU_OUT] * 7
        den = [0] * 7
        for lane, v in delay_map.items():
            delay[lane] = v
            den[lane] = 1
        return UopDpConfig(
            op=op, alu_src0=src0, alu_src1=src1, delay=delay,
            alu_out_enable=1, swap_enable=0, alu_out_a_enable=out_a,
            alu_out_b_enable=0, delay_enable=den, idx0_sel=0, idx1_sel=0,
        )

    u0, u1 = copy.deepcopy(uops[0]), copy.deepcopy(uops[1])
    for u in (u0, u1):
        u.inp = [InpSel.ZERO, InpSel.SRC_0, InpSel.SRC_1, InpSel.SRC_0_HI,
                 InpSel.SRC_1_HI, InpSel.CONST_0, InpSel.ZERO, InpSel.ZERO]
        u.inp_enable = [0, 1, 1, 1, 1, 1, 0, 0]
    # seed: route C0 (lane4) down the ALU chain into block3's flop (the
    # steady state's accumulator register)
    u0.datapath_config = [
        dp(BYP, A.PREV_DELAY_4, A.PREV_DELAY_4, {4: PD}),
        dp(BYP, A.PREV_ALU_OUT, A.PREV_ALU_OUT, {}),
        dp(BYP, A.PREV_ALU_OUT, A.PREV_ALU_OUT, {}),
        dp(BYP, A.PREV_ALU_OUT, A.PREV_ALU_OUT, {}, out_a=1),
        dp(BYP, A.PREV_ALU_OUT, A.PREV_ALU_OUT, {}, out_a=1),
        dp(BYP, A.PREV_ALU_OUT, A.PREV_ALU_OUT, {}, out_a=1),
        dp(BYP, A.PREV_ALU_OUT, A.PREV_ALU_OUT, {}, out_a=1),
        dp(BYP, A.PREV_ALU_OUT, A.PREV_ALU_OUT, {}, out_a=1),
    ]
    # steady: lanes at entry: 0=E0 1=E1 2=O0 3=O1 4=C0
    u1.datapath_config = [
        dp(MAX, A.PREV_DELAY_0, A.PREV_DELAY_1, {0: PD, 2: PD, 3: PD}),
        dp(MAX, A.PREV_DELAY_2, A.PREV_DELAY_3, {0: PD, 1: PAO, 2: PD}),
        dp(MAX, A.PREV_DELAY_0, A.PREV_DELAY_2, {1: PD, 2: PAO}),
        dp(MAX, A.CURR_ALU_OUT, A.PREV_ALU_OUT, {1: PD, 2: PD}, out_a=1),
        dp(BYP, A.PREV_ALU_OUT, A.PREV_ALU_OUT, {1: PD, 2: PD}, out_a=1),
        dp(BYP, A.PREV_ALU_OUT, A.PREV_ALU_OUT, {1: PD, 2: PD}, out_a=1),
        dp(BYP, A.PREV_ALU_OUT, A.PREV_ALU_OUT, {1: PD, 2: PD}, out_a=1),
        dp(BYP, A.PREV_ALU_OUT, A.PREV_ALU_OUT, {1: PD, 2: PD}, out_a=1),
    ]
    u1.out = {OutPath.WR0_LO: OutSel.DELAY_1, OutPath.WR0_HI: OutSel.DELAY_2,
              OutPath.WR1_LO: OutSel.ALU_OUT, OutPath.WR1_HI: OutSel.ALU_OUT}
    u1.out_enable = {OutPath.WR0_LO: 1, OutPath.WR0_HI: 1,
                     OutPath.WR1_LO: 0, OutPath.WR1_HI: 0}
    return [u0, u1]


def _get_maxboth_op():
    import concourse.dve_ops as dve_ops_mod
    from concourse.dve_ops import DveOp
    from concourse.dve_spec import Spec, Src0, Src1, C0, maxx, lower
    from concourse.dve_uop import DveOpSpec, AluInp

    if _MAXBOTH_NAME in dve_ops_mod._SUB_OPCODE_FOR_NAME:
        for op in dve_ops_mod.OPS:
            if op.name == _MAXBOTH_NAME:
                return op

    def _ref(in0, in1, c0, c1, c2):
        out = np.maximum(in0.astype(np.float32), in1.astype(np.float32))
        acc = np.maximum(
            in0.astype(np.float32)
            .reshape(in0.shape[0], -1)
            .max(axis=-1, keepdims=True),
            c0,
        )
        return out, acc

    spec = Spec(body=maxx(Src0, Src1), accum=maxx, accum_init=C0,
                reference=_ref)
    row = max(dve_ops_mod._SUB_OPCODE_FOR_NAME.values()) + 1
    assert row < 0x20
    dve_ops_mod._SUB_OPCODE_FOR_NAME[_MAXBOTH_NAME] = row
    shas = {}
    for ver in ("v3", "v4"):
        uops = lower(spec, ver=ver)
        # accumulator stage: fold Src0, not the body (probe-validated)
        blk = uops[1].datapath_config[1]
        assert blk.alu_src0 == AluInp.CURR_ALU_OUT and blk.alu_out_a_enable
        assert blk.alu_src1 == AluInp.PREV_ALU_OUT
        blk.alu_src1 = AluInp.PREV_DELAY_0
        dspec = DveOpSpec(name=_MAXBOTH_NAME, opcode=row, uops=uops,
                          uops_2x=_make_2x(uops), perf_max=1, rd1_en=True)
        dspec.validate(ver)
        shas[ver] = dspec.sha(ver)
        dve_ops_mod._COMPILE_CACHE[(_MAXBOTH_NAME, ver)] = dspec
    op = DveOp(_MAXBOTH_NAME, spec, subdim=False, uops_sha=shas)
    dve_ops_mod.OPS.append(op)
    dve_ops_mod.CUSTOM_DVE_SPECS[_MAXBOTH_NAME] = spec
    return op


_PROGS = {}


def _build_program(repeat=1):
    from concourse import bass, bacc, tile, mybir

    f32 = mybir.dt.float32
    bf = mybir.dt.bfloat16

    nc = bacc.Bacc("TRN2", target_bir_lowering=False, debug=False)
    ab_d = nc.declare_dram_parameter("ab", [56, GR, N], bf, isOutput=False)
    id_d = nc.declare_dram_parameter("ident", [PT, PT], f32, isOutput=False)
    out_d = nc.declare_dram_parameter("out", [1, 1], f32, isOutput=True)

    maxboth = _get_maxboth_op()
    NH = N // CH               # psum tiles per row-block
    SL = CH // PT              # transpose slots per psum tile
    NEG = -3.0e38

    with tile.TileContext(nc) as tc:
        with (
            tc.tile_pool(name="io", bufs=1) as io_pool,
            tc.tile_pool(name="sb", bufs=3) as sb_pool,
            tc.tile_pool(name="ps", bufs=2, space=bass.MemorySpace.PSUM) as ps_pool,
            tc.tile_pool(name="misc", bufs=1) as misc_pool,
        ):
            abt = io_pool.tile([56, GR, N], bf)
            for g in range(GR):
                nc.sync.dma_start(out=abt[:, g, :], in_=ab_d[:, g, :])
            ident = io_pool.tile([PT, PT], f32)
            nc.sync.dma_start(out=ident[:], in_=id_d[:])
            identb = io_pool.tile([PT, PT], bf)
            nc.scalar.copy(out=identb[:], in_=ident[:])

            # running colmax ping-pong (bf16, shared by both paths),
            # per-(b,i) rowmax accs, per-column minima
            cm = misc_pool.tile([PT, 2, N], bf, tag="cm")
            acc = misc_pool.tile([PT, NH, BPC * NI], f32, tag="acc")
            colred = misc_pool.tile([PT, BPC * NI], f32, tag="colred")

            qg = 0  # alternating PE row group
            for rep in range(repeat):
              for b in range(BPC):
                nc.gpsimd.memset(cm[:, 0, :], NEG)
                for i in range(NI):
                    col = b * NI + i
                    cur, nxt = i % 2, (i + 1) % 2
                    bf16_path = i in BF16_BLOCKS
                    if bf16_path:
                        bt = sb_pool.tile([PT, N], bf, tag="bt")
                    else:
                        bt = None
                    for h in range(NH):
                        ps3 = ps_pool.tile([PT, SL, PT], f32, tag="ps")
                        for q in range(CH // FT):
                            r0 = 32 * qg
                            qg ^= 1
                            nc.tensor.matmul(
                                ps3[:, q * (FT // PT):(q + 1) * (FT // PT), :],
                                abt[r0:r0 + KAUG, 2 * b,
                                    i * PT:(i + 1) * PT],
                                abt[r0:r0 + KAUG, 2 * b + 1,
                                    h * CH + q * FT:h * CH + (q + 1) * FT],
                                start=True, stop=True,
                            )
                        if bf16_path:
                            nc.scalar.copy(
                                out=bt[:, h * CH:(h + 1) * CH],
                                in_=ps3[:, :, :],
                            )
                        else:
                            nc.vector._custom_dve(
                                maxboth,
                                out=cm[:, nxt, h * CH:(h + 1) * CH],
                                in0=ps3[:, :, :],
                                in1=cm[:, cur, h * CH:(h + 1) * CH],
                                s0=(NEG if h == 0
                                    else acc[:, h - 1, col:col + 1]),
                                accum_out=acc[:, h, col:col + 1],
                            )
                    if bf16_path:
                        # 1x only: the DVE 2X_1PORT accumulator readout is
                        # firmware-incomplete (custom-dve T1) — the body
                        # runs at 2x but accum_out returns garbage, so the
                        # op must stay at 1x for the rowmax side.
                        nc.vector._custom_dve(
                            maxboth,
                            out=cm[:, nxt, :],
                            in0=bt[:, :],
                            in1=cm[:, cur, :],
                            s0=NEG,
                            accum_out=acc[:, NH - 1, col:col + 1],
                        )
                # col-side fold: final colmax is in cm[:, 0, :] (NI even);
                # PE-transpose the bf16 colmax directly (bf16 identity),
                # per-column max via TR — chunked so transpose/reduce
                # pipeline across t.
                for t in range(N // CH):
                    pst = ps_pool.tile([PT, SL, PT], bf, tag="ps")
                    for k in range(SL):
                        nc.tensor.transpose(
                            pst[:, k, :],
                            cm[:, 0, t * CH + k * PT:t * CH + (k + 1) * PT],
                            identb[:],
                        )
                    nc.vector.tensor_reduce(
                        out=colred[:, b * NI + t * SL:b * NI + (t + 1) * SL],
                        in_=pst[:, :, :],
                        axis=mybir.AxisListType.X, op=mybir.AluOpType.max,
                    )

            # total = sum(row accs) + sum(col minima); partitions folded by a
            # K=128 matmul against ones.
            rtot = misc_pool.tile([PT, 1], f32, tag="rtot")
            nc.vector.tensor_reduce(
                out=rtot[:], in_=acc[:, NH - 1, :],
                axis=mybir.AxisListType.X, op=mybir.AluOpType.add,
            )
            ctot = misc_pool.tile([PT, 1], f32, tag="ctot")
            nc.vector.tensor_reduce(
                out=ctot[:], in_=colred[:, :],
                axis=mybir.AxisListType.X, op=mybir.AluOpType.add,
            )
            nc.vector.tensor_tensor(
                out=rtot[:], in0=rtot[:], in1=ctot[:],
                op=mybir.AluOpType.add,
            )
            ones = misc_pool.tile([PT, 1], f32, tag="ones")
            nc.gpsimd.memset(ones[:], 1.0)
            psc = ps_pool.tile([1, 1], f32, tag="ps")
            nc.tensor.matmul(psc[:], rtot[:], ones[:], start=True, stop=True)
            res = misc_pool.tile([1, 1], f32, tag="res")
            nc.vector.tensor_copy(res[:], psc[:])
            nc.sync.dma_start(out=out_d[:], in_=res[:])

    nc.compile()
    return nc


def get_program(repeat=1):
    if repeat not in _PROGS:
        _PROGS[repeat] = _build_program(repeat)
    return _PROGS[repeat]


def _hml(x):
    """3-level bf16 split: x ~= h + m + l to ~2^-27 relative."""
    h = x.astype(BF16)
    r1 = x - h.astype(np.float32)
    m = r1.astype(BF16)
    l = (r1 - m.astype(np.float32)).astype(BF16)
    return h, m, l


def _sides(a_pts, b_pts):
    """a_pts: stationary [n,3] fp32; b_pts: moving [m,3] fp32.

    Returns (A [KAUG,n], Bm [KAUG,m]) bf16 with A^T @ Bm == NEGATED pairwise
    squared distances (device maxes of -dist are mins of dist)."""
    n, m = len(a_pts), len(b_pts)
    sqa = np.sum(a_pts * a_pts, axis=-1, dtype=np.float32)
    sqb = np.sum(b_pts * b_pts, axis=-1, dtype=np.float32)
    bm = (-2.0 * b_pts).astype(np.float32)
    A = np.zeros((KAUG, n), BF16)
    Bm = np.zeros((KAUG, m), BF16)
    for d in range(D):
        ah, am, al = _hml(a_pts[:, d])
        bh, bmid, bl = _hml(bm[:, d])
        for s, (av, bv) in enumerate(
            [(ah, bh), (ah, bmid), (am, bh), (ah, bl), (al, bh), (am, bmid)]
        ):
            A[6 * d + s] = av
            Bm[6 * d + s] = bv
    sh, sm, sl = _hml(sqa)
    A[18], A[19], A[20] = sh, sm, sl
    Bm[18] = Bm[19] = Bm[20] = 1
    sh, sm, sl = _hml(sqb)
    A[21] = A[22] = A[23] = 1
    Bm[21], Bm[22], Bm[23] = sh, sm, sl
    return A, -Bm


def build_inputs(p1, p2):
    """Per-core device input tensors: [NCORES][56, GR, N] bf16.
    Rows 0:24 and 32:56 are identical copies (PE row-group alternation)."""
    ab = np.zeros((NCORES, 56, GR, N), BF16)
    for c in range(NCORES):
        for b in range(BPC):
            gb = c * BPC + b
            A1, B1 = _sides(p1[gb], p2[gb])
            ab[c, 0:KAUG, 2 * b + 0] = A1
            ab[c, 0:KAUG, 2 * b + 1] = B1
        ab[c, 32:32 + KAUG] = ab[c, 0:KAUG]
    return ab


def run_cores(ab, trace=False, repeat=1):
    """Run the SPMD program over 8 cores; returns (partials [NCORES], results)."""
    from concourse.bass_utils import run_bass_kernel_spmd

    nc = get_program(repeat)
    in_maps = [
        {"ab": np.ascontiguousarray(ab[c]), "ident": IDENT}
        for c in range(NCORES)
    ]
    res = run_bass_kernel_spmd(nc, in_maps, list(range(NCORES)), trace=trace)
    # device sums max(-dist) per core; negate to get the chamfer partial
    partials = np.array(
        [-np.float64(res.results[c]["out"][0, 0]) for c in range(NCORES)]
    )
    return partials, res


def kernel(points1, points2):
    p1 = np.asarray(points1, dtype=np.float32)
    p2 = np.asarray(points2, dtype=np.float32)
    ab = build_inputs(p1, p2)
    last_err = None
    for attempt in range(3):
        try:
            partials, _ = run_cores(ab, trace=False)
            return np.array(partials.sum() / B, dtype=np.float32)
        except Exception as e:  # transient NRT exec-unit wedge recovers on retry
            last_err = e
            time.sleep(2.0)
    raise last_err

